# revision 35
# baseline (speedup 1.0000x reference)
"""ALIF/LIF spiking recurrence on 8 TRN2 NeuronCores.

Recurrence (over time dim 0 of x[T=100, B=128, N=4096], f32):
    mem_t = mem_{t-1} * 0.2 * (1 - spk_{t-1}) + x_t
    spk_t = (mem_t > 0.5).astype(f32)
Output: spk [T, B, N] f32.

Strategy: shard N across the 8 cores (512 columns each, data parallel —
the recurrence is elementwise so no collectives). Per core the membrane
state lives in SBUF and each timestep is ONE custom fused DVE micro-op
(registered at runtime into the per-NEFF uop table):

    mem_t = select(0.5 >= mem_{t-1}, mem_{t-1}, 0) * 0.2 + x_t

which is bit-identical in rounding to the reference recurrence. The
spike output is produced on the otherwise-idle ScalarE, once per
10-step slab, as Sign(mem - 0.5) written to uint8 (-1 saturates to 0) —
exactly (mem > 0.5), including the mem == 0.5 edge case — quartering
the store traffic. Input x streams in 2.5MB slabs on the SP HWDGE ring
(5-deep prefetch); u8 spike slabs stream out on the ACT HWDGE ring so
stores never head-of-line block input slabs. Measured ~109-114us on
silicon: ~91us of DMA at the ~358 GB/s per-core HBM roofline (26.2MB in
+ 6.5MB out per core) plus ~18us fixed NEFF preamble/tail; DVE busy is
~71us and hidden under the DMA stream.
"""

import os
import sys

import numpy as np

for _p in ("/opt/trn_rl_repo", "/root/.axon_site/_ro/trn_rl_repo"):
    if _p not in sys.path and os.path.isdir(_p):
        sys.path.insert(0, _p)

import concourse.bass as bass
import concourse.dve_ops as dve_ops
import concourse.tile as tile
from concourse import bacc, mybir
from concourse.bass_utils import run_bass_kernel_spmd
from concourse.dve_spec import C0, C1, Spec, Src0, Src1, Zero, _has_src1, lower, select
from concourse.dve_uop import DveOpSpec

T, B, N = 100, 128, 4096
NCORES = 8
NS = N // NCORES  # 512 columns per core
DECAY = 0.2
THRESH = 0.5

F32 = mybir.dt.float32
U8 = mybir.dt.uint8
Op = mybir.AluOpType

# timesteps per input/output DMA slab: small head slabs so DVE starts
# ~1us into the run instead of after a full 2.5MB slab DMA; small tail
# slabs so the drain (last ACT + out-DMA) is short.
SLABS = [2, 8] + [10] * 8 + [6, 4]
assert sum(SLABS) == T

LAST_RESULTS = None  # set by kernel(); test.py reads exec_time_ns from here


def _register_alif_op():
    """Register a custom fused DVE op computing one full ALIF step:

        out = select(0.5 >= in0, in0, 0) * 0.2 + in1
            = mem_prev * (mem_prev <= 0.5) * DECAY + x_t

    One DVE instruction per timestep (vs two scalar_tensor_tensor ops),
    bit-identical rounding to the reference. The op is appended to
    dve_ops.OPS at runtime; the per-NEFF DVE uop table is generated from
    OPS at compile time, so no firmware/repo change is needed.
    """
    if "ALIF_STEP" in dve_ops._SUB_OPCODE_FOR_NAME:
        return next(o for o in dve_ops.OPS if o.name == "ALIF_STEP")
    spec = Spec(
        body=select(C1 >= Src0, Src0, Zero) * C0 + Src1,
        reference=lambda in0, in1, s0, s1, imm2: (
            np.where(np.float32(s1) >= in0, in0, np.float32(0.0)).astype(np.float32)
            * np.float32(s0)
            + in1
        ).astype(np.float32),
    )
    row = dve_ops._CUSTOM_DVE_ROW_BASE + len(dve_ops.OPS)
    shas = {}
    for ver in ("v3", "v4"):
        shas[ver] = DveOpSpec(
            name="ALIF_STEP", opcode=row, uops=lower(spec, ver=ver),
            rd1_en=_has_src1(spec),
        ).sha(ver)
    op = dve_ops.DveOp("ALIF_STEP", spec, subdim=False, uops_sha=shas)
    dve_ops.OPS.append(op)
    dve_ops._SUB_OPCODE_FOR_NAME[op.name] = row
    dve_ops.CUSTOM_DVE_SPECS[op.name] = spec
    return op


ALIF_OP = _register_alif_op()


def build_nc() -> bass.Bass:
    # Bacc (not raw Bass): its compile() runs generate_event_semaphores,
    # which splits multi-wait instructions to satisfy the TRN2 "at most
    # one sync wait per instruction" constraint.
    nc = bacc.Bacc()
    x = nc.declare_dram_parameter("x", [T, B, NS], F32, isOutput=False)
    out = nc.declare_dram_parameter("out", [T, B, NS], U8, isOutput=True)

    # const AP for the activation bias (Sign needs bias as an SBUF AP);
    # the memset is issued inside the TileContext so Tile orders the
    # activations after it without an explicit all-engine barrier.
    bias_t = nc.alloc_sbuf_tensor(f"const-float32--0.5", [128, 1], F32)
    nc.const_aps.aps[(F32, -THRESH)] = bias_t.ap()

    with tile.TileContext(nc) as tc:
        nc.vector.memset(bias_t.ap(), -THRESH)
        with (
            tc.tile_pool(name="xs", bufs=5) as xpool,
            tc.tile_pool(name="os", bufs=3) as opool,
            tc.tile_pool(name="mem", bufs=3) as mpool,
        ):
            prev = None
            t0 = 0
            for slab in SLABS:
                xt = xpool.tile([B, slab, NS], F32, tag="xs")
                nc.sync.dma_start(xt[:], x[t0 : t0 + slab].rearrange("t p n -> p t n"))
                ot = opool.tile([B, slab, NS], U8, tag="os")
                # mem for the whole slab lives in one tile so the spike
                # activation runs once per slab (one ScalarE op + one
                # cross-engine WAR instead of ten).
                ms = mpool.tile([B, slab, NS], F32, tag="ms")
                for s in range(slab):
                    if prev is None:
                        # mem_0 = x_0 (initial state is zero)
                        nc.vector.tensor_copy(ms[:, s, :], xt[:, s, :])
                    else:
                        # one fused DVE op: mem = (prev<=0.5)*prev*0.2 + x_t
                        nc.vector._custom_dve(
                            ALIF_OP,
                            out=ms[:, s, :],
                            in0=prev,
                            in1=xt[:, s, :],
                            s0=DECAY,
                            s1=THRESH,
                        )
                    prev = ms[:, s, :]
                # spike: Sign(mem - 0.5) -> u8 saturates {-1,0,1} to {0,0,1}
                nc.scalar.activation(
                    ot[:].rearrange("p t n -> p (t n)"),
                    ms[:].rearrange("p t n -> p (t n)"),
                    mybir.ActivationFunctionType.Sign,
                    bias=-THRESH,
                    scale=1.0,
                )
                # out-DMAs ride the ACT HWDGE ring (qActDynamicHW) so they
                # never head-of-line block input slabs on the SP ring.
                nc.scalar.dma_start(
                    out[t0 : t0 + slab].rearrange("t p n -> p t n"), ot[:]
                )
                t0 += slab
    nc.finalize()
    return nc


def make_in_maps(x_np: np.ndarray) -> list[dict]:
    return [
        {"x": np.ascontiguousarray(x_np[:, :, i * NS : (i + 1) * NS])}
        for i in range(NCORES)
    ]


def assemble_out(results: list[dict]) -> np.ndarray:
    shards = [np.asarray(results[i]["out"]) for i in range(NCORES)]
    return np.concatenate(shards, axis=2).astype(np.float32)


def kernel(x) -> np.ndarray:
    global LAST_RESULTS
    x_np = np.asarray(x, dtype=np.float32)
    assert x_np.shape == (T, B, N), x_np.shape

    nc = build_nc()
    res = run_bass_kernel_spmd(
        nc, make_in_maps(x_np), core_ids=list(range(NCORES))
    )
    LAST_RESULTS = res
    return assemble_out(res.results)


if __name__ == "__main__":
    rng = np.random.default_rng(0)
    xt = rng.standard_normal((T, B, N), dtype=np.float32)
    y = kernel(xt)
    print("out", y.shape, y.dtype, "mean spike rate", y.mean())


# revision 36
# speedup vs baseline: 1.0214x; 1.0214x over previous
"""ALIF/LIF spiking recurrence on 8 TRN2 NeuronCores.

Recurrence (over time dim 0 of x[T=100, B=128, N=4096], f32):
    mem_t = mem_{t-1} * 0.2 * (1 - spk_{t-1}) + x_t
    spk_t = (mem_t > 0.5).astype(f32)
Output: spk [T, B, N] f32.

Strategy: shard N across the 8 cores (512 columns each, data parallel —
the recurrence is elementwise so no collectives). Per core the membrane
state lives in SBUF and each timestep is ONE custom fused DVE micro-op
(registered at runtime into the per-NEFF uop table):

    mem_t = select(0.5 >= mem_{t-1}, mem_{t-1}, 0) * 0.2 + x_t

which is bit-identical in rounding to the reference recurrence. The
spike output is produced on the otherwise-idle ScalarE, once per
10-step slab, as Sign(mem - 0.5) written to uint8 (-1 saturates to 0) —
exactly (mem > 0.5), including the mem == 0.5 edge case — quartering
the store traffic. Input x streams in 2.5MB slabs on the SP HWDGE ring
(5-deep prefetch); u8 spike slabs stream out on the ACT HWDGE ring so
stores never head-of-line block input slabs. Measured ~109-114us on
silicon: ~91us of DMA at the ~358 GB/s per-core HBM roofline (26.2MB in
+ 6.5MB out per core) plus ~18us fixed NEFF preamble/tail; DVE busy is
~71us and hidden under the DMA stream.
"""

import os
import sys

import numpy as np

for _p in ("/opt/trn_rl_repo", "/root/.axon_site/_ro/trn_rl_repo"):
    if _p not in sys.path and os.path.isdir(_p):
        sys.path.insert(0, _p)

import concourse.bass as bass
import concourse.dve_ops as dve_ops
import concourse.tile as tile
from concourse import bacc, mybir
from concourse.bass_utils import run_bass_kernel_spmd
from concourse.dve_spec import C0, C1, Spec, Src0, Src1, Zero, _has_src1, lower, select
from concourse.dve_uop import DveOpSpec

T, B, N = 100, 128, 4096
NCORES = 8
NS = N // NCORES  # 512 columns per core
DECAY = 0.2
THRESH = 0.5

F32 = mybir.dt.float32
U8 = mybir.dt.uint8
Op = mybir.AluOpType

# timesteps per input/output DMA slab: small head slabs so DVE starts
# ~1us into the run instead of after a full 2.5MB slab DMA; small tail
# slabs so the drain (last ACT + out-DMA) is short.
SLABS = [2, 8] + [10] * 8 + [6, 4]
assert sum(SLABS) == T

LAST_RESULTS = None  # set by kernel(); test.py reads exec_time_ns from here


def _register_alif_op():
    """Register a custom fused DVE op computing one full ALIF step:

        out = select(0.5 >= in0, in0, 0) * 0.2 + in1
            = mem_prev * (mem_prev <= 0.5) * DECAY + x_t

    One DVE instruction per timestep (vs two scalar_tensor_tensor ops),
    bit-identical rounding to the reference. The op is appended to
    dve_ops.OPS at runtime; the per-NEFF DVE uop table is generated from
    OPS at compile time, so no firmware/repo change is needed.
    """
    if "ALIF_STEP" in dve_ops._SUB_OPCODE_FOR_NAME:
        return next(o for o in dve_ops.OPS if o.name == "ALIF_STEP")
    spec = Spec(
        body=select(C1 >= Src0, Src0, Zero) * C0 + Src1,
        reference=lambda in0, in1, s0, s1, imm2: (
            np.where(np.float32(s1) >= in0, in0, np.float32(0.0)).astype(np.float32)
            * np.float32(s0)
            + in1
        ).astype(np.float32),
    )
    row = dve_ops._CUSTOM_DVE_ROW_BASE + len(dve_ops.OPS)
    shas = {}
    for ver in ("v3", "v4"):
        shas[ver] = DveOpSpec(
            name="ALIF_STEP", opcode=row, uops=lower(spec, ver=ver),
            rd1_en=_has_src1(spec),
        ).sha(ver)
    op = dve_ops.DveOp("ALIF_STEP", spec, subdim=False, uops_sha=shas)
    dve_ops.OPS.append(op)
    dve_ops._SUB_OPCODE_FOR_NAME[op.name] = row
    dve_ops.CUSTOM_DVE_SPECS[op.name] = spec
    return op


ALIF_OP = _register_alif_op()


def build_nc() -> bass.Bass:
    # Bacc (not raw Bass): its compile() runs generate_event_semaphores,
    # which splits multi-wait instructions to satisfy the TRN2 "at most
    # one sync wait per instruction" constraint.
    nc = bacc.Bacc()
    x = nc.declare_dram_parameter("x", [T, B, NS], F32, isOutput=False)
    out = nc.declare_dram_parameter("out", [T, B, NS], U8, isOutput=True)

    # const AP for the activation bias (Sign needs bias as an SBUF AP);
    # the memset is issued inside the TileContext so Tile orders the
    # activations after it without an explicit all-engine barrier.
    bias_t = nc.alloc_sbuf_tensor(f"const-float32--0.5", [128, 1], F32)
    nc.const_aps.aps[(F32, -THRESH)] = bias_t.ap()

    with tile.TileContext(nc) as tc:
        nc.vector.memset(bias_t.ap(), -THRESH)
        with (
            tc.tile_pool(name="xs", bufs=6) as xpool,
            tc.tile_pool(name="os", bufs=3) as opool,
            tc.tile_pool(name="mem", bufs=2) as mpool,
        ):
            prev = None
            t0 = 0
            for slab in SLABS:
                xt = xpool.tile([B, slab, NS], F32, tag="xs")
                nc.sync.dma_start(xt[:], x[t0 : t0 + slab].rearrange("t p n -> p t n"))
                ot = opool.tile([B, slab, NS], U8, tag="os")
                # mem for the whole slab lives in one tile so the spike
                # activation runs once per slab (one ScalarE op + one
                # cross-engine WAR instead of ten).
                ms = mpool.tile([B, slab, NS], F32, tag="ms")
                for s in range(slab):
                    if prev is None:
                        # mem_0 = x_0 (initial state is zero)
                        nc.vector.tensor_copy(ms[:, s, :], xt[:, s, :])
                    else:
                        # one fused DVE op: mem = (prev<=0.5)*prev*0.2 + x_t
                        nc.vector._custom_dve(
                            ALIF_OP,
                            out=ms[:, s, :],
                            in0=prev,
                            in1=xt[:, s, :],
                            s0=DECAY,
                            s1=THRESH,
                        )
                    prev = ms[:, s, :]
                # spike: Sign(mem - 0.5) -> u8 saturates {-1,0,1} to {0,0,1}
                nc.scalar.activation(
                    ot[:].rearrange("p t n -> p (t n)"),
                    ms[:].rearrange("p t n -> p (t n)"),
                    mybir.ActivationFunctionType.Sign,
                    bias=-THRESH,
                    scale=1.0,
                )
                # out-DMAs ride the ACT HWDGE ring (qActDynamicHW) so they
                # never head-of-line block input slabs on the SP ring.
                nc.scalar.dma_start(
                    out[t0 : t0 + slab].rearrange("t p n -> p t n"), ot[:]
                )
                t0 += slab
    nc.finalize()
    return nc


def make_in_maps(x_np: np.ndarray) -> list[dict]:
    return [
        {"x": np.ascontiguousarray(x_np[:, :, i * NS : (i + 1) * NS])}
        for i in range(NCORES)
    ]


def assemble_out(results: list[dict]) -> np.ndarray:
    shards = [np.asarray(results[i]["out"]) for i in range(NCORES)]
    return np.concatenate(shards, axis=2).astype(np.float32)


def kernel(x) -> np.ndarray:
    global LAST_RESULTS
    x_np = np.asarray(x, dtype=np.float32)
    assert x_np.shape == (T, B, N), x_np.shape

    nc = build_nc()
    res = run_bass_kernel_spmd(
        nc, make_in_maps(x_np), core_ids=list(range(NCORES))
    )
    LAST_RESULTS = res
    return assemble_out(res.results)


if __name__ == "__main__":
    rng = np.random.default_rng(0)
    xt = rng.standard_normal((T, B, N), dtype=np.float32)
    y = kernel(xt)
    print("out", y.shape, y.dtype, "mean spike rate", y.mean())
